# revision 13
# baseline (speedup 1.0000x reference)
"""MoE feed-forward (8 experts, top-2, SwiGLU) on 8 Trainium2 NeuronCores.

Strategy: routed expert parallelism. Core c owns expert c and computes the
SwiGLU expert output only for the tokens routed to it (host gathers them
compactly, feature-major). Router top-2 softmax gating is computed exactly
on host; the per-token gating prob is shipped as an input and applied on
device. Host scatter-adds the compact per-core outputs into the full
[B, S, D] result.

Compute is bf16 (weights + activations; fp32 PSUM accumulate): the PE
streams 1 column/cycle for both fp32r and bf16, but bf16 halves DMA bytes
and, via fast-weight-load, halves LDWEIGHTS so the matmul stream — not the
weight loads — is the limiter. All device inputs are host-packed so every
DMA is one contiguous run per partition (128 descriptors — descriptor
generation is ~4ns/descriptor on the issuing engine and would otherwise
serialize the head). Inputs stream on two HWDGE queues (weights on sync,
activations on scalar), outputs go out on the vector queue so the sync
engine's end-of-kernel semaphore-reset chain isn't blocked behind the last
output transfer. A few memset-fed warmup matmuls at program start lift the
PE clock gate (HAM) to full rate before real data lands. Measured
end-to-end rel-err ~4e-3 (gate: 2e-2).
"""

import os
import sys
import time

sys.path.insert(0, "/opt/trn_rl_repo")

import numpy as np

# ---------------------------------------------------------------------------
# Problem constants (hardcoded per contract)
B, S, D, E, I, TOPK = 2, 2048, 1024, 8, 1408, 2
T = B * S  # 4096 tokens
P = 128
D_T = D // P   # 8 d-tiles
I_T = I // P   # 11 i-tiles
N_CORES = 8
N_WARMUP = 20  # dummy MMs bridging the DMA-limited head so HAM stays hot
# i-tile blocks for the wg/wu weight streams (pipelined arrival; first
# blocks small so the i_o=0/1 matmuls unblock as early as possible)
WBLOCKS = [(0, 1), (1, 3), (3, 5), (5, 7), (7, 9), (9, 11)]

_VERBOSE = bool(int(os.environ.get("KERNEL_VERBOSE", "0")))


def _log(msg):
    if _VERBOSE:
        print(f"[kernel] {msg}", flush=True)


def round_f32r(a: np.ndarray) -> np.ndarray:
    """RNE-round fp32 array to 13 mantissa bits (FP22 / e8m13)."""
    v = np.ascontiguousarray(a, dtype=np.float32).view(np.uint32)
    low = v & np.uint32(0x1FFF)
    base = v & np.uint32(0xFFFFE000)
    lsb = (v >> np.uint32(13)) & np.uint32(1)
    round_up = (low > np.uint32(0x1000)) | ((low == np.uint32(0x1000)) & (lsb == 1))
    out = base + (round_up.astype(np.uint32) << np.uint32(13))
    return out.view(np.float32)


def host_gating(x2d: np.ndarray, gate_w: np.ndarray):
    """Exact router: scores -> top-2 -> softmax. Returns gating [T, E] fp32."""
    scores = x2d.astype(np.float64) @ gate_w.astype(np.float64).T  # [T, E]
    idx = np.argsort(-scores, axis=-1, kind="stable")[:, :TOPK]  # [T, 2]
    top = np.take_along_axis(scores, idx, axis=-1)  # [T, 2] descending
    m = top[:, :1]
    ex = np.exp(top - m)
    probs = ex / ex.sum(axis=-1, keepdims=True)  # [T, 2]
    gating = np.zeros((x2d.shape[0], E), dtype=np.float64)
    np.put_along_axis(gating, idx, probs, axis=-1)
    return gating.astype(np.float32)


def pack_rows(a3: np.ndarray) -> np.ndarray:
    """[G, P, W] -> [P, G*W]: per-partition-contiguous device layout."""
    return np.ascontiguousarray(a3.transpose(1, 0, 2).reshape(P, -1))


# ---------------------------------------------------------------------------
# Bass kernel builder


def build_nc_routed(cap, tc, n_cores=N_CORES):
    """Each core computes its expert's SwiGLU for `cap` host-gathered tokens
    (feature-major), scales by the gating prob, writes compact [D, cap]."""
    import concourse.mybir as mybir
    import concourse.tile as tile
    from concourse import bacc

    f32 = mybir.dt.float32
    f32r = mybir.dt.float32r
    bf16 = mybir.dt.bfloat16
    n_chunks = cap // tc
    assert cap % tc == 0

    nc = bacc.Bacc("TRN2", debug=False, num_devices=n_cores)

    # Packed inputs: one contiguous run per partition for every DMA.
    # xc_p[dp, ci*D_T*tc + do*tc + t], w*_p[dp, blk-major (do, i)], see
    # make_in_maps_routed.
    xc_d = nc.dram_tensor("xc", [P, n_chunks * D_T * tc], bf16,
                          kind="ExternalInput")
    wg_d = nc.dram_tensor("wg", [P, D_T * I], bf16, kind="ExternalInput")
    wu_d = nc.dram_tensor("wu", [P, D_T * I], bf16, kind="ExternalInput")
    wd_d = nc.dram_tensor("wd", [P, I_T * D], bf16, kind="ExternalInput")
    gprob_d = nc.dram_tensor("gprob", [1, cap], bf16, kind="ExternalInput")
    ycomp_d = nc.dram_tensor("ycomp", [D, cap], f32, kind="ExternalOutput")

    ycomp_r = ycomp_d.ap().rearrange("(do dp) t -> dp do t", dp=P)

    # flat offsets for the block-major wg/wu layout
    blk_of_tile = {}
    blk_off = {}
    off = 0
    for b, (s0, s1) in enumerate(WBLOCKS):
        blk_off[b] = off
        for i_o in range(s0, s1):
            blk_of_tile[i_o] = (b, i_o - s0)
        off += D_T * (s1 - s0) * P
    assert off == D_T * I

    def w_slice(w_sb, d_o, i_o):
        b, j = blk_of_tile[i_o]
        s0, s1 = WBLOCKS[b]
        bw = (s1 - s0) * P
        o = blk_off[b] + d_o * bw + j * P
        return w_sb[:, o:o + P]

    with tile.TileContext(nc) as tcx:
        with tcx.tile_pool(name="wpool", bufs=1) as wpool, \
             tcx.tile_pool(name="xpool", bufs=n_chunks) as xpool, \
             tcx.tile_pool(name="hpool", bufs=2) as hpool, \
             tcx.tile_pool(name="ypool", bufs=2) as ypool, \
             tcx.tile_pool(name="gspool", bufs=3) as gspool, \
             tcx.tile_pool(name="gbpool", bufs=2) as gbpool, \
             tcx.tile_pool(name="psg", bufs=2, space="PSUM") as psg, \
             tcx.tile_pool(name="psu", bufs=2, space="PSUM") as psu, \
             tcx.tile_pool(name="psy", bufs=2, space="PSUM") as psy, \
             tcx.tile_pool(name="psb", bufs=1, space="PSUM") as psb:

            # ---- consts: memset, no DMA ----
            # full 128-partition warmup operand: K=1 matmuls barely register
            # on the PE activity monitor, so warmups must contract over all
            # 128 rows to lift the clock gate
            warm_c = wpool.tile([P, 512], bf16)
            nc.gpsimd.memset(warm_c[:], 1.0)

            # PE warmup: lift the HAM clock gate before real data lands
            warm_ps = psb.tile([P, 512], f32, tag="gbps", name="warm")
            for _ in range(N_WARMUP):
                nc.tensor.matmul(warm_ps[:], warm_c[:, :P], warm_c[:],
                                 start=True, stop=True)

            wg_sb = wpool.tile([P, D_T * I], bf16)
            wu_sb = wpool.tile([P, D_T * I], bf16)
            wd_sb = wpool.tile([P, I_T * D], bf16)
            gprob_sb = wpool.tile([1, cap], bf16)

            xts = []
            for ci in range(n_chunks):
                xts.append(xpool.tile([P, D_T * tc], bf16, tag="xt",
                                      name=f"xt{ci}"))

            # two parallel HWDGE queues: per-group gate weights (sync) and
            # up weights + activations (scalar) arrive concurrently
            xw = D_T * tc
            def blk_rng(b):
                s0, s1 = WBLOCKS[b]
                return blk_off[b], blk_off[b] + D_T * (s1 - s0) * P

            for s in range(0, D_T, 4):  # chunk 0 in d_o quads
                nc.scalar.dma_start(xts[0][:, s * tc:(s + 4) * tc],
                                    xc_d.ap()[:, s * tc:(s + 4) * tc])
            for b in (0, 1, 2):
                o0, o1 = blk_rng(b)
                nc.scalar.dma_start(wu_sb[:, o0:o1], wu_d.ap()[:, o0:o1])
            nc.scalar.dma_start(gprob_sb[:], gprob_d.ap())
            if n_chunks > 1:
                nc.scalar.dma_start(xts[1][:], xc_d.ap()[:, xw:2 * xw])
            for b in range(3, len(WBLOCKS)):
                o0, o1 = blk_rng(b)
                nc.scalar.dma_start(wu_sb[:, o0:o1], wu_d.ap()[:, o0:o1])
            for ci in range(2, n_chunks):
                nc.scalar.dma_start(xts[ci][:],
                                    xc_d.ap()[:, ci * xw:(ci + 1) * xw])

            for b in range(len(WBLOCKS)):
                o0, o1 = blk_rng(b)
                nc.sync.dma_start(wg_sb[:, o0:o1], wg_d.ap()[:, o0:o1])
            half = I_T * D // 2
            nc.sync.dma_start(wd_sb[:, :half], wd_d.ap()[:, :half])
            nc.sync.dma_start(wd_sb[:, half:], wd_d.ap()[:, half:])

            for ci in range(n_chunks):
                t0 = ci * tc
                xt = xts[ci]

                h = hpool.tile([P, I_T, tc], bf16, tag="h")
                for i_o in range(I_T):
                    pg = psg.tile([P, tc], f32, tag="pg")
                    pu = psu.tile([P, tc], f32, tag="pu")
                    for d_o in range(D_T):
                        nc.tensor.matmul(
                            pg[:], w_slice(wg_sb, d_o, i_o),
                            xt[:, d_o * tc:(d_o + 1) * tc],
                            start=(d_o == 0), stop=(d_o == D_T - 1))
                    for d_o in range(D_T):
                        nc.tensor.matmul(
                            pu[:], w_slice(wu_sb, d_o, i_o),
                            xt[:, d_o * tc:(d_o + 1) * tc],
                            start=(d_o == 0), stop=(d_o == D_T - 1))
                    gs = gspool.tile([P, tc], bf16, tag="gs")
                    nc.scalar.activation(gs[:], pg[:],
                                         mybir.ActivationFunctionType.Silu)
                    nc.vector.tensor_mul(out=h[:, i_o, :], in0=gs[:],
                                         in1=pu[:])

                # broadcast this chunk's gating row to 128 partitions just
                # before the down-proj that consumes it
                gb_ps = psb.tile([P, tc], f32, tag="gbps")
                nc.tensor.matmul(gb_ps[:], warm_c[:1, :P],
                                 gprob_sb[:, t0:t0 + tc],
                                 start=True, stop=True)
                gb_sb = gbpool.tile([P, tc], f32, tag="gb")
                nc.vector.tensor_copy(out=gb_sb[:], in_=gb_ps[:])

                yout = ypool.tile([P, D_T, tc], f32, tag="yout")
                for d_o in range(D_T):
                    py = psy.tile([P, tc], f32, tag="py")
                    for i_o in range(I_T):
                        nc.tensor.matmul(
                            py[:], wd_sb[:, i_o * D + d_o * P:
                                         i_o * D + d_o * P + P],
                            h[:, i_o, :],
                            start=(i_o == 0), stop=(i_o == I_T - 1))
                    nc.vector.tensor_mul(out=yout[:, d_o, :], in0=py[:],
                                         in1=gb_sb[:])
                # outputs on the scalar HWDGE queue (fast descriptor gen;
                # keeps the sync engine's end-of-kernel semaphore resets off
                # the critical path)
                ostep = 1 if ci == n_chunks - 1 else 2
                for s0 in range(0, D_T, ostep):
                    nc.scalar.dma_start(
                        ycomp_r[:, s0:s0 + ostep, t0:t0 + tc],
                        yout[:, s0:s0 + ostep, :])

    nc.compile()
    return nc


# ---------------------------------------------------------------------------
# Host-side wrapper

_CACHED = {}


def _get_nc_routed(cap, tc, n_cores=N_CORES):
    key = ("routed", cap, tc, n_cores)
    if key not in _CACHED:
        t0 = time.time()
        _CACHED[key] = build_nc_routed(cap, tc, n_cores)
        _log(f"built routed bass program (cap={cap}, tc={tc}) in "
             f"{time.time() - t0:.1f}s")
    return _CACHED[key]


def _round_up(v, m):
    return (v + m - 1) // m * m


def make_in_maps_routed(x, gate_w, gate_proj_w, up_proj_w, down_proj_w):
    """Returns (in_maps, idx_list, n_list, cap, tc)."""
    from concurrent.futures import ThreadPoolExecutor
    import ml_dtypes

    bf16 = ml_dtypes.bfloat16
    x2d = np.ascontiguousarray(np.asarray(x, np.float32).reshape(T, D))
    gating = host_gating(x2d, np.asarray(gate_w, np.float32))  # [T, E]
    idx_list = [np.nonzero(gating[:, c] > 0)[0].astype(np.int64)
                for c in range(N_CORES)]
    n_list = [len(ix) for ix in idx_list]
    # Fewest chunks with width <= 512 (PSUM bank limit), evenly sized.
    max_n = max(n_list)
    n_chunks = max(1, -(-max_n // 512))
    tc = _round_up(-(-max_n // n_chunks), 4)
    cap = tc * n_chunks

    xT_bf = np.ascontiguousarray(x2d.T).astype(bf16)  # [D, T]

    def pack_w(w, blocks=None):
        # w: [I, D] or [D, I] source -> device flat [P, G*W] block-major
        a3 = w.reshape(-1, P, w.shape[-1])  # [G, P, W]
        if blocks is None:
            return pack_rows(a3).astype(bf16)
        parts = [pack_rows(a3[:, :, s0 * P:s1 * P]) for s0, s1 in blocks]
        return np.concatenate(parts, axis=1).astype(bf16)

    def prep_core(c):
        ix, n_c = idx_list[c], n_list[c]
        xcT = np.zeros((D, cap), dtype=bf16)
        xcT[:, :n_c] = xT_bf[:, ix]
        # pack x chunk-major: [P, ci*D_T*tc + do*tc + t]
        x3 = xcT.reshape(D_T, P, cap)
        xc_p = np.concatenate(
            [pack_rows(x3[:, :, ci * tc:(ci + 1) * tc])
             for ci in range(n_chunks)], axis=1)
        gprob = np.zeros((1, cap), dtype=np.float32)
        gprob[0, :n_c] = gating[ix, c]
        wgT = np.asarray(gate_proj_w[c], np.float32).T  # [D, I]
        wuT = np.asarray(up_proj_w[c], np.float32).T    # [D, I]
        wdT = np.asarray(down_proj_w[c], np.float32).T  # [I, D]
        return {
            "xc": xc_p,
            "wg": pack_w(wgT, WBLOCKS),
            "wu": pack_w(wuT, WBLOCKS),
            "wd": pack_w(wdT),
            "gprob": gprob.astype(bf16),
        }

    with ThreadPoolExecutor(N_CORES) as ex:
        in_maps = list(ex.map(prep_core, range(N_CORES)))
    return in_maps, idx_list, n_list, cap, tc


def kernel(x, gate_w, gate_proj_w, up_proj_w, down_proj_w,
           num_experts_per_tok=2, _trace=False, _trace_cores=None):
    from concourse import bass_utils
    assert int(num_experts_per_tok) == TOPK

    kwargs = {}
    if _trace:
        try:
            sys.path.insert(0, os.path.dirname(os.path.abspath(__file__)))
            import axon_profile_shim
            axon_profile_shim.install()
        except Exception as exc:  # profiling is best-effort
            _log(f"profile shim unavailable: {exc}")
        kwargs = dict(trace=True,
                      trace_cores=_trace_cores or list(range(N_CORES)))

    t0 = time.time()
    in_maps, idx_list, n_list, cap, tc = make_in_maps_routed(
        x, gate_w, gate_proj_w, up_proj_w, down_proj_w)
    _log(f"host prep {time.time() - t0:.1f}s (cap={cap}, tc={tc}, "
         f"counts={n_list})")
    nc = _get_nc_routed(cap, tc)
    t0 = time.time()
    res = bass_utils.run_bass_kernel_spmd(
        nc, in_maps, core_ids=list(range(N_CORES)), **kwargs)
    _log(f"run_bass_kernel_spmd took {time.time() - t0:.1f}s")
    kernel.last_result = res
    t0 = time.time()
    y = np.zeros((T, D), dtype=np.float32)
    for c in range(N_CORES):
        yc = res.results[c]["ycomp"]  # [D, cap]
        y[idx_list[c]] += np.ascontiguousarray(yc[:, :n_list[c]].T)
    _log(f"host combine {time.time() - t0:.1f}s")
    return y.reshape(B, S, D)


kernel.last_result = None


# revision 14
# speedup vs baseline: 1.0239x; 1.0239x over previous
"""MoE feed-forward (8 experts, top-2, SwiGLU) on 8 Trainium2 NeuronCores.

Strategy: routed expert parallelism. Core c owns expert c and computes the
SwiGLU expert output only for the tokens routed to it (host gathers them
compactly, feature-major). Router top-2 softmax gating is computed exactly
on host; the per-token gating prob is shipped as an input and applied on
device. Host scatter-adds the compact per-core outputs into the full
[B, S, D] result.

Compute is bf16 (weights + activations; fp32 PSUM accumulate): the PE
streams 1 column/cycle for both fp32r and bf16, but bf16 halves DMA bytes
and, via fast-weight-load, halves LDWEIGHTS so the matmul stream — not the
weight loads — is the limiter. All device inputs are host-packed so every
DMA is one contiguous run per partition (128 descriptors — descriptor
generation is ~4ns/descriptor on the issuing engine and would otherwise
serialize the head). Inputs stream on two HWDGE queues (weights on sync,
activations on scalar), outputs go out on the vector queue so the sync
engine's end-of-kernel semaphore-reset chain isn't blocked behind the last
output transfer. A few memset-fed warmup matmuls at program start lift the
PE clock gate (HAM) to full rate before real data lands. Measured
end-to-end rel-err ~4e-3 (gate: 2e-2).
"""

import os
import sys
import time

sys.path.insert(0, "/opt/trn_rl_repo")

import numpy as np

# ---------------------------------------------------------------------------
# Problem constants (hardcoded per contract)
B, S, D, E, I, TOPK = 2, 2048, 1024, 8, 1408, 2
T = B * S  # 4096 tokens
P = 128
D_T = D // P   # 8 d-tiles
I_T = I // P   # 11 i-tiles
N_CORES = 8
N_WARMUP = 8   # dummy MMs filling the pre-supply window (PE idle anyway)
# i-tile blocks for the wg/wu weight streams (pipelined arrival; first
# blocks small so the i_o=0/1 matmuls unblock as early as possible)
WBLOCKS = [(0, 1), (1, 3), (3, 5), (5, 7), (7, 9), (9, 11)]

_VERBOSE = bool(int(os.environ.get("KERNEL_VERBOSE", "0")))


def _log(msg):
    if _VERBOSE:
        print(f"[kernel] {msg}", flush=True)


def round_f32r(a: np.ndarray) -> np.ndarray:
    """RNE-round fp32 array to 13 mantissa bits (FP22 / e8m13)."""
    v = np.ascontiguousarray(a, dtype=np.float32).view(np.uint32)
    low = v & np.uint32(0x1FFF)
    base = v & np.uint32(0xFFFFE000)
    lsb = (v >> np.uint32(13)) & np.uint32(1)
    round_up = (low > np.uint32(0x1000)) | ((low == np.uint32(0x1000)) & (lsb == 1))
    out = base + (round_up.astype(np.uint32) << np.uint32(13))
    return out.view(np.float32)


def host_gating(x2d: np.ndarray, gate_w: np.ndarray):
    """Exact router: scores -> top-2 -> softmax. Returns gating [T, E] fp32."""
    scores = x2d.astype(np.float64) @ gate_w.astype(np.float64).T  # [T, E]
    idx = np.argsort(-scores, axis=-1, kind="stable")[:, :TOPK]  # [T, 2]
    top = np.take_along_axis(scores, idx, axis=-1)  # [T, 2] descending
    m = top[:, :1]
    ex = np.exp(top - m)
    probs = ex / ex.sum(axis=-1, keepdims=True)  # [T, 2]
    gating = np.zeros((x2d.shape[0], E), dtype=np.float64)
    np.put_along_axis(gating, idx, probs, axis=-1)
    return gating.astype(np.float32)


def pack_rows(a3: np.ndarray) -> np.ndarray:
    """[G, P, W] -> [P, G*W]: per-partition-contiguous device layout."""
    return np.ascontiguousarray(a3.transpose(1, 0, 2).reshape(P, -1))


# ---------------------------------------------------------------------------
# Bass kernel builder


def build_nc_routed(cap, tc, n_cores=N_CORES):
    """Each core computes its expert's SwiGLU for `cap` host-gathered tokens
    (feature-major), scales by the gating prob, writes compact [D, cap]."""
    import concourse.mybir as mybir
    import concourse.tile as tile
    from concourse import bacc

    f32 = mybir.dt.float32
    f32r = mybir.dt.float32r
    bf16 = mybir.dt.bfloat16
    n_chunks = cap // tc
    assert cap % tc == 0

    nc = bacc.Bacc("TRN2", debug=False, num_devices=n_cores)

    # Packed inputs: one contiguous run per partition for every DMA.
    # xc_p[dp, ci*D_T*tc + do*tc + t], w*_p[dp, blk-major (do, i)], see
    # make_in_maps_routed.
    xc_d = nc.dram_tensor("xc", [P, n_chunks * D_T * tc], bf16,
                          kind="ExternalInput")
    wg_d = nc.dram_tensor("wg", [P, D_T * I], bf16, kind="ExternalInput")
    wu_d = nc.dram_tensor("wu", [P, D_T * I], bf16, kind="ExternalInput")
    wd_d = nc.dram_tensor("wd", [P, I_T * D], bf16, kind="ExternalInput")
    gprob_d = nc.dram_tensor("gprob", [1, cap], bf16, kind="ExternalInput")
    ycomp_d = nc.dram_tensor("ycomp", [D, cap], f32, kind="ExternalOutput")

    ycomp_r = ycomp_d.ap().rearrange("(do dp) t -> dp do t", dp=P)

    # flat offsets for the block-major wg/wu layout
    blk_of_tile = {}
    blk_off = {}
    off = 0
    for b, (s0, s1) in enumerate(WBLOCKS):
        blk_off[b] = off
        for i_o in range(s0, s1):
            blk_of_tile[i_o] = (b, i_o - s0)
        off += D_T * (s1 - s0) * P
    assert off == D_T * I

    def w_slice(w_sb, d_o, i_o):
        b, j = blk_of_tile[i_o]
        s0, s1 = WBLOCKS[b]
        bw = (s1 - s0) * P
        o = blk_off[b] + d_o * bw + j * P
        return w_sb[:, o:o + P]

    with tile.TileContext(nc) as tcx:
        with tcx.tile_pool(name="wpool", bufs=1) as wpool, \
             tcx.tile_pool(name="xpool", bufs=n_chunks) as xpool, \
             tcx.tile_pool(name="hpool", bufs=2) as hpool, \
             tcx.tile_pool(name="ypool", bufs=2) as ypool, \
             tcx.tile_pool(name="gspool", bufs=3) as gspool, \
             tcx.tile_pool(name="gbpool", bufs=2) as gbpool, \
             tcx.tile_pool(name="psg", bufs=2, space="PSUM") as psg, \
             tcx.tile_pool(name="psu", bufs=2, space="PSUM") as psu, \
             tcx.tile_pool(name="psy", bufs=2, space="PSUM") as psy, \
             tcx.tile_pool(name="psb", bufs=1, space="PSUM") as psb:

            # ---- consts: memset, no DMA ----
            # full 128-partition warmup operand: K=1 matmuls barely register
            # on the PE activity monitor, so warmups must contract over all
            # 128 rows to lift the clock gate
            warm_c = wpool.tile([P, 512], bf16)
            nc.gpsimd.memset(warm_c[:], 1.0)

            # PE warmup: lift the HAM clock gate before real data lands
            warm_ps = psb.tile([P, 512], f32, tag="gbps", name="warm")
            for _ in range(N_WARMUP):
                nc.tensor.matmul(warm_ps[:], warm_c[:, :P], warm_c[:],
                                 start=True, stop=True)

            wg_sb = wpool.tile([P, D_T * I], bf16)
            wu_sb = wpool.tile([P, D_T * I], bf16)
            wd_sb = wpool.tile([P, I_T * D], bf16)
            gprob_sb = wpool.tile([1, cap], bf16)

            xts = []
            for ci in range(n_chunks):
                xts.append(xpool.tile([P, D_T * tc], bf16, tag="xt",
                                      name=f"xt{ci}"))

            # two parallel HWDGE queues: per-group gate weights (sync) and
            # up weights + activations (scalar) arrive concurrently
            xw = D_T * tc
            def blk_rng(b):
                s0, s1 = WBLOCKS[b]
                return blk_off[b], blk_off[b] + D_T * (s1 - s0) * P

            for s in range(0, D_T, 4):  # chunk 0 in d_o quads
                nc.scalar.dma_start(xts[0][:, s * tc:(s + 4) * tc],
                                    xc_d.ap()[:, s * tc:(s + 4) * tc])
            for b in (0, 1, 2):
                o0, o1 = blk_rng(b)
                nc.scalar.dma_start(wu_sb[:, o0:o1], wu_d.ap()[:, o0:o1])
            nc.scalar.dma_start(gprob_sb[:], gprob_d.ap())
            if n_chunks > 1:
                nc.scalar.dma_start(xts[1][:], xc_d.ap()[:, xw:2 * xw])
            for b in range(3, len(WBLOCKS)):
                o0, o1 = blk_rng(b)
                nc.scalar.dma_start(wu_sb[:, o0:o1], wu_d.ap()[:, o0:o1])
            for ci in range(2, n_chunks):
                nc.scalar.dma_start(xts[ci][:],
                                    xc_d.ap()[:, ci * xw:(ci + 1) * xw])

            for b in range(len(WBLOCKS)):
                o0, o1 = blk_rng(b)
                nc.sync.dma_start(wg_sb[:, o0:o1], wg_d.ap()[:, o0:o1])
            half = I_T * D // 2
            nc.sync.dma_start(wd_sb[:, :half], wd_d.ap()[:, :half])
            nc.sync.dma_start(wd_sb[:, half:], wd_d.ap()[:, half:])

            for ci in range(n_chunks):
                t0 = ci * tc
                xt = xts[ci]

                h = hpool.tile([P, I_T, tc], bf16, tag="h")
                for i_o in range(I_T):
                    pg = psg.tile([P, tc], f32, tag="pg")
                    pu = psu.tile([P, tc], f32, tag="pu")
                    for d_o in range(D_T):
                        nc.tensor.matmul(
                            pg[:], w_slice(wg_sb, d_o, i_o),
                            xt[:, d_o * tc:(d_o + 1) * tc],
                            start=(d_o == 0), stop=(d_o == D_T - 1))
                    for d_o in range(D_T):
                        nc.tensor.matmul(
                            pu[:], w_slice(wu_sb, d_o, i_o),
                            xt[:, d_o * tc:(d_o + 1) * tc],
                            start=(d_o == 0), stop=(d_o == D_T - 1))
                    gs = gspool.tile([P, tc], bf16, tag="gs")
                    nc.scalar.activation(gs[:], pg[:],
                                         mybir.ActivationFunctionType.Silu)
                    nc.vector.tensor_mul(out=h[:, i_o, :], in0=gs[:],
                                         in1=pu[:])

                # broadcast this chunk's gating row to 128 partitions just
                # before the down-proj that consumes it
                gb_ps = psb.tile([P, tc], f32, tag="gbps")
                nc.tensor.matmul(gb_ps[:], warm_c[:1, :P],
                                 gprob_sb[:, t0:t0 + tc],
                                 start=True, stop=True)
                gb_sb = gbpool.tile([P, tc], f32, tag="gb")
                nc.vector.tensor_copy(out=gb_sb[:], in_=gb_ps[:])

                yout = ypool.tile([P, D_T, tc], f32, tag="yout")
                for d_o in range(D_T):
                    py = psy.tile([P, tc], f32, tag="py")
                    for i_o in range(I_T):
                        nc.tensor.matmul(
                            py[:], wd_sb[:, i_o * D + d_o * P:
                                         i_o * D + d_o * P + P],
                            h[:, i_o, :],
                            start=(i_o == 0), stop=(i_o == I_T - 1))
                    nc.vector.tensor_mul(out=yout[:, d_o, :], in0=py[:],
                                         in1=gb_sb[:])
                # outputs on the scalar HWDGE queue (fast descriptor gen;
                # keeps the sync engine's end-of-kernel semaphore resets off
                # the critical path)
                ostep = 1 if ci == n_chunks - 1 else 2
                for s0 in range(0, D_T, ostep):
                    nc.scalar.dma_start(
                        ycomp_r[:, s0:s0 + ostep, t0:t0 + tc],
                        yout[:, s0:s0 + ostep, :])

    nc.compile()
    return nc


# ---------------------------------------------------------------------------
# Host-side wrapper

_CACHED = {}


def _get_nc_routed(cap, tc, n_cores=N_CORES):
    key = ("routed", cap, tc, n_cores)
    if key not in _CACHED:
        t0 = time.time()
        _CACHED[key] = build_nc_routed(cap, tc, n_cores)
        _log(f"built routed bass program (cap={cap}, tc={tc}) in "
             f"{time.time() - t0:.1f}s")
    return _CACHED[key]


def _round_up(v, m):
    return (v + m - 1) // m * m


def make_in_maps_routed(x, gate_w, gate_proj_w, up_proj_w, down_proj_w):
    """Returns (in_maps, idx_list, n_list, cap, tc)."""
    from concurrent.futures import ThreadPoolExecutor
    import ml_dtypes

    bf16 = ml_dtypes.bfloat16
    x2d = np.ascontiguousarray(np.asarray(x, np.float32).reshape(T, D))
    gating = host_gating(x2d, np.asarray(gate_w, np.float32))  # [T, E]
    idx_list = [np.nonzero(gating[:, c] > 0)[0].astype(np.int64)
                for c in range(N_CORES)]
    n_list = [len(ix) for ix in idx_list]
    # Fewest chunks with width <= 512 (PSUM bank limit), evenly sized.
    max_n = max(n_list)
    n_chunks = max(1, -(-max_n // 512))
    tc = _round_up(-(-max_n // n_chunks), 4)
    cap = tc * n_chunks

    xT_bf = np.ascontiguousarray(x2d.T).astype(bf16)  # [D, T]

    def pack_w(w, blocks=None):
        # w: [I, D] or [D, I] source -> device flat [P, G*W] block-major
        a3 = w.reshape(-1, P, w.shape[-1])  # [G, P, W]
        if blocks is None:
            return pack_rows(a3).astype(bf16)
        parts = [pack_rows(a3[:, :, s0 * P:s1 * P]) for s0, s1 in blocks]
        return np.concatenate(parts, axis=1).astype(bf16)

    def prep_core(c):
        ix, n_c = idx_list[c], n_list[c]
        xcT = np.zeros((D, cap), dtype=bf16)
        xcT[:, :n_c] = xT_bf[:, ix]
        # pack x chunk-major: [P, ci*D_T*tc + do*tc + t]
        x3 = xcT.reshape(D_T, P, cap)
        xc_p = np.concatenate(
            [pack_rows(x3[:, :, ci * tc:(ci + 1) * tc])
             for ci in range(n_chunks)], axis=1)
        gprob = np.zeros((1, cap), dtype=np.float32)
        gprob[0, :n_c] = gating[ix, c]
        wgT = np.asarray(gate_proj_w[c], np.float32).T  # [D, I]
        wuT = np.asarray(up_proj_w[c], np.float32).T    # [D, I]
        wdT = np.asarray(down_proj_w[c], np.float32).T  # [I, D]
        return {
            "xc": xc_p,
            "wg": pack_w(wgT, WBLOCKS),
            "wu": pack_w(wuT, WBLOCKS),
            "wd": pack_w(wdT),
            "gprob": gprob.astype(bf16),
        }

    with ThreadPoolExecutor(N_CORES) as ex:
        in_maps = list(ex.map(prep_core, range(N_CORES)))
    return in_maps, idx_list, n_list, cap, tc


def kernel(x, gate_w, gate_proj_w, up_proj_w, down_proj_w,
           num_experts_per_tok=2, _trace=False, _trace_cores=None):
    from concourse import bass_utils
    assert int(num_experts_per_tok) == TOPK

    kwargs = {}
    if _trace:
        try:
            sys.path.insert(0, os.path.dirname(os.path.abspath(__file__)))
            import axon_profile_shim
            axon_profile_shim.install()
        except Exception as exc:  # profiling is best-effort
            _log(f"profile shim unavailable: {exc}")
        kwargs = dict(trace=True,
                      trace_cores=_trace_cores or list(range(N_CORES)))

    t0 = time.time()
    in_maps, idx_list, n_list, cap, tc = make_in_maps_routed(
        x, gate_w, gate_proj_w, up_proj_w, down_proj_w)
    _log(f"host prep {time.time() - t0:.1f}s (cap={cap}, tc={tc}, "
         f"counts={n_list})")
    nc = _get_nc_routed(cap, tc)
    t0 = time.time()
    res = bass_utils.run_bass_kernel_spmd(
        nc, in_maps, core_ids=list(range(N_CORES)), **kwargs)
    _log(f"run_bass_kernel_spmd took {time.time() - t0:.1f}s")
    kernel.last_result = res
    t0 = time.time()
    y = np.zeros((T, D), dtype=np.float32)
    for c in range(N_CORES):
        yc = res.results[c]["ycomp"]  # [D, cap]
        y[idx_list[c]] += np.ascontiguousarray(yc[:, :n_list[c]].T)
    _log(f"host combine {time.time() - t0:.1f}s")
    return y.reshape(B, S, D)


kernel.last_result = None
